# revision 36
# baseline (speedup 1.0000x reference)
"""Causal linear attention (elu+1 feature map) for Trainium2, 8-core SPMD.

Sharding: core c = (a, b) with a = c//4 (batch index) and b = c%4 (head
quarter: heads [4b:4b+4) of 16, i.e. feature columns [256b:256b+256)).

Pipeline per core (all matmuls bf16, fp32 PSUM):
  loads: fp32->bf16 cast-DMAs on Pool, ordered k, q, v (l-sliced chunks so
      consumers fire as slices land), Wo last; mask/identity constants are
      host-fed over SP so Pool starts immediately.
  P1: k'/q' feature-major projections + phi(x)=elu(x)+1 (bf16 temps, one
      full-width max per region); l-major k via PE transposes; natural v
      with an appended ones column (for the denominator); binary-prefix
      state blocks S in two 2-pair PSUM tiles.
  P2: per (pair, chunk) O'^T (128 l, 65) = A'^T V^ + Q'^T S_prefix;
      division by the denominator column is a per-partition scalar op
      (DVE/Act alternating); divided chunks stay l-major.
  A2A: ONE 8-way AllToAll (512KB, l-major payload; shard j = l-chunk j).
      A PE transpose/copy "keeper" chain spans the collective so the
      tensor engine p-state stays ramped.
  P3: receiver transposes the payload back to feature-major (PE), then
      projects with Wo; 4 PSUM groups stream to DRAM via SBUF copies.

Emission interleaves P1c/P2 so the PE never idles behind program order;
PSUM pools follow a strict LIFO lifecycle (t2/a2/sh2/pj2 -> o shares a's
banks -> p3).

Host side only slices/transposes/replicates numpy inputs (pure layout) and
reassembles the sharded outputs.
"""

import sys

sys.path.insert(0, "/opt/trn_rl_repo")

from contextlib import ExitStack

import numpy as np

import concourse.bass as bass
import concourse.mybir as mybir
from concourse.tile import TileContext

F32 = mybir.dt.float32
BF16 = mybir.dt.bfloat16

L = 1024          # sequence length
NB = 2            # batch
E = 1024          # embed dim
H = 16            # heads
D = 64            # head dim
EPS = 1e-6
N_CORES = 8
FPC = 256         # features per core (4 heads)
C = 128           # chunk size
NCH = L // C      # chunks per (head, batch) pair

LAST_RESULT = None  # set by kernel() for test harnesses


def _split_waits(nc, cap=1):
    """Walrus allows only one sync-wait on pseudo instructions (DMA triggers,
    collective triggers, drains). Move excess waits onto preceding single-wait
    NoOps on the same engine (engine FIFO order keeps semantics identical)."""
    ctr = 0
    for f in nc.m.functions:
        for blk in f.blocks:
            insts = list(blk.instructions)
            new = []
            changed = False
            for ins in insts:
                si = ins.sync_info
                waits = list(si.on_wait) if (si and si.on_wait) else []
                if len(waits) > cap:
                    excess, keep = waits[:-cap], waits[-cap:]
                    for w in excess:
                        new.append(mybir.InstNoOp(
                            name=f"wsplit-{ctr}",
                            sync_info=mybir.SyncInfo(on_wait=[w], on_update=[]),
                            engine=ins.engine,
                            bass_nofuse=True,
                        ))
                        ctr += 1
                    ins.sync_info = mybir.SyncInfo(on_wait=keep, on_update=si.on_update)
                    changed = True
                new.append(ins)
            if changed:
                blk.instructions = new
    return ctr


def _build(with_bias):
    nc = bass.Bass(num_devices=N_CORES)

    xqT = nc.declare_dram_parameter("xqT", [E, L], F32, isOutput=False)
    xkT = nc.declare_dram_parameter("xkT", [E, L], F32, isOutput=False)
    xvT = nc.declare_dram_parameter("xvT", [E, L], F32, isOutput=False)
    wqT = nc.declare_dram_parameter("wqT", [E, FPC], F32, isOutput=False)
    wkT = nc.declare_dram_parameter("wkT", [E, FPC], F32, isOutput=False)
    wvT = nc.declare_dram_parameter("wvT", [E, FPC], F32, isOutput=False)
    woT = nc.declare_dram_parameter("woT", [E, E], F32, isOutput=False)
    mask_d = nc.declare_dram_parameter("maskc", [C, 4 * C], F32, isOutput=False)
    id_d = nc.declare_dram_parameter("identc", [C, C], BF16, isOutput=False)
    if with_bias:
        bq_d = nc.declare_dram_parameter("bq", [FPC, 1], F32, isOutput=False)
        bk_d = nc.declare_dram_parameter("bk", [FPC, 1], F32, isOutput=False)
        bv_d = nc.declare_dram_parameter("bv", [FPC, 1], F32, isOutput=False)
        bo_d = nc.declare_dram_parameter("bo", [1, E], F32, isOutput=False)
    out_d = nc.declare_dram_parameter("out", [NB, C, E], F32, isOutput=True)
    tok_i = nc.declare_dram_parameter("tok", [1, 1], F32, isOutput=False)
    tok_o = nc.declare_dram_parameter("tok_out", [1, 1], F32, isOutput=True)

    with TileContext(nc) as tc:
        es = ExitStack()
        constp = es.enter_context(tc.tile_pool(name="const", bufs=1))
        xTp = es.enter_context(tc.tile_pool(name="xT", bufs=1))
        wTp = es.enter_context(tc.tile_pool(name="wT", bufs=1))
        woTp = es.enter_context(tc.tile_pool(name="woTp", bufs=1))
        projp = es.enter_context(tc.tile_pool(name="proj", bufs=1))
        tmpp = es.enter_context(tc.tile_pool(name="tmp", bufs=3))
        p2p = es.enter_context(tc.tile_pool(name="p2", bufs=3))
        atTp = es.enter_context(tc.tile_pool(name="atT", bufs=1))
        outp = es.enter_context(tc.tile_pool(name="outp", bufs=1))
        dram = es.enter_context(tc.tile_pool(name="dram", bufs=1, space="DRAM"))
        # PSUM pools -- 8 banks total, strict LIFO lifecycle:
        # [t, a, sh] live long; pj (P1 projections) closes after vnat, then o
        # (P2 O'^T) opens in its zone; o closes before p3 opens.
        ps_t = es.enter_context(tc.tile_pool(name="ps_t", bufs=2, space="PSUM"))
        ps_a = es.enter_context(tc.tile_pool(name="ps_a", bufs=2, space="PSUM"))
        ps_sh = es.enter_context(
            tc.tile_pool(name="ps_sh", bufs=2, space="PSUM"))
        pj_stack = ExitStack()
        ps_pj = pj_stack.enter_context(
            tc.tile_pool(name="ps_pj", bufs=2, space="PSUM"))

        with es:
            # ---------------- constants (host-fed, via SP so Pool can
            # start the big loads immediately) ----------------
            mask2 = constp.tile([C, 4 * C], F32)   # 4x upper-tri incl diag
            nc.sync.dma_start(out=mask2[:], in_=mask_d[:])
            ident = constp.tile([C, C], BF16)      # for PE transposes
            nc.sync.dma_start(out=ident[:], in_=id_d[:])

            if with_bias:
                bqs = [constp.tile([C, 1], F32, tag=f"bq{m}", name=f"bqs{m}") for m in range(2)]
                bks = [constp.tile([C, 1], F32, tag=f"bk{m}", name=f"bks{m}") for m in range(2)]
                bq1 = [constp.tile([C, 1], F32, tag=f"bq1{m}", name=f"bq1{m}") for m in range(2)]
                bk1 = [constp.tile([C, 1], F32, tag=f"bk1{m}", name=f"bk1{m}") for m in range(2)]
                for m in range(2):
                    nc.sync.dma_start(out=bqs[m][:], in_=bq_d[m * C:(m + 1) * C])
                    nc.sync.dma_start(out=bks[m][:], in_=bk_d[m * C:(m + 1) * C])
                    nc.vector.tensor_scalar(bq1[m][:], bqs[m][:], 1.0, None,
                                            op0=mybir.AluOpType.add)
                    nc.vector.tensor_scalar(bk1[m][:], bks[m][:], 1.0, None,
                                            op0=mybir.AluOpType.add)
                bo_row = constp.tile([1, E], F32)
                nc.sync.dma_start(out=bo_row[:], in_=bo_d[:])
                bvr = constp.tile([1, FPC], F32)
                nc.sync.dma_start(out=bvr[:], in_=bv_d[:].rearrange("f one -> one f"))
                ones_row1 = constp.tile([1, C], F32)
                nc.vector.memset(ones_row1[:], 1.0)
                bo_bc = constp.tile([C, E], F32)
                for nbk in range(2):
                    bo_ps = ps_pj.tile([C, 512], F32, tag="pj")
                    nc.tensor.matmul(bo_ps[:, 0:512], ones_row1[:, 0:C],
                                     bo_row[:, nbk * 512:(nbk + 1) * 512],
                                     start=True, stop=True)
                    nc.vector.tensor_copy(bo_bc[:, nbk * 512:(nbk + 1) * 512],
                                          bo_ps[:])
                bv_bc = constp.tile([C, FPC], F32)
                bps = ps_pj.tile([C, FPC], F32, tag="pj")
                nc.tensor.matmul(bps[:], ones_row1[:, 0:C], bvr[:],
                                 start=True, stop=True)
                nc.vector.tensor_copy(bv_bc[:], bps[:])

            # ---------------- input loads (cast fp32 -> bf16) ----------------
            def big_load(pool, src, kd, fd, nm, chunks=1, l_chunks=0):
                t = pool.tile([C, kd, fd], BF16, tag=nm, name=nm)
                src_r = src[:].rearrange("(k p) f -> p k f", p=C)
                if l_chunks:
                    step = fd // l_chunks
                    for h in range(l_chunks):
                        fs = slice(h * step, (h + 1) * step)
                        nc.gpsimd.dma_start(out=t[:, :, fs],
                                            in_=src_r[:, :, fs])
                    return t
                step = kd // chunks
                for h in range(chunks):
                    ks = slice(h * step, (h + 1) * step)
                    nc.gpsimd.dma_start(out=t[:, ks, :], in_=src_r[:, ks, :])
                return t

            wk_all = big_load(wTp, wkT, 8, FPC, "wk_all")
            xk_all = big_load(xTp, xkT, 8, L, "xk_all", l_chunks=4)
            wq_all = big_load(wTp, wqT, 8, FPC, "wq_all")
            xq_all = big_load(xTp, xqT, 8, L, "xq_all", l_chunks=4)
            wv_all = big_load(wTp, wvT, 8, FPC, "wv_all")
            xv_all = big_load(xTp, xvT, 8, L, "xv_all", l_chunks=4)
            wo_all = big_load(woTp, woT, 8, E, "wo_all", chunks=4)
            xk_sb = [xk_all[:, k, :] for k in range(8)]
            xq_sb = [xq_all[:, k, :] for k in range(8)]
            xv_sb = [xv_all[:, k, :] for k in range(8)]
            wk_sb = [wk_all[:, k, :] for k in range(8)]
            wq_sb = [wq_all[:, k, :] for k in range(8)]
            wv_sb = [wv_all[:, k, :] for k in range(8)]
            wo_sb = [wo_all[:, k, :] for k in range(8)]

            # ---------------- P1a: feature-major k' / q' ----------------
            # half tiles (128, L) bf16: rows [0:64)=pair 2m, [64:128)=pair 2m+1
            kp_h = [projp.tile([C, L], BF16, tag=f"kp{m}", name=f"kp{m}")
                    for m in range(2)]
            qp_h = [projp.tile([C, L], BF16, tag=f"qp{m}", name=f"qp{m}")
                    for m in range(2)]

            def kp_sl(p, cs):
                return kp_h[p // 2][(p % 2) * D:(p % 2) * D + D, cs]

            def qp_sl(p, cs):
                return qp_h[p // 2][(p % 2) * D:(p % 2) * D + D, cs]

            def proj_phi(which, wsb, xsb, dst, m, nbk):
                ps = ps_pj.tile([C, 512], F32, tag="pj")
                for k in range(8):
                    nc.tensor.matmul(
                        ps[:], wsb[k][:, m * C:(m + 1) * C],
                        xsb[k][:, nbk * 512:(nbk + 1) * 512],
                        start=(k == 0), stop=(k == 7))
                cs = slice(nbk * 512, (nbk + 1) * 512)
                # phi(x) = max(exp(min(x,0)), x+1); bf16 temps let the final
                # max run in the DVE 2x perf mode.
                tu = tmpp.tile([C, 512], BF16, tag="tu")
                if with_bias:
                    bias1 = (bq1 if which == "q" else bk1)[m][:]
                    nc.vector.tensor_scalar(tu[:], ps[:], bias1, None,
                                            op0=mybir.AluOpType.add)
                else:
                    nc.vector.tensor_scalar(tu[:], ps[:], 1.0, None,
                                            op0=mybir.AluOpType.add)
                tmin = tmpp.tile([C, 512], BF16, tag="tmin")
                nc.scalar.activation(tmin[:], tu[:],
                                     mybir.ActivationFunctionType.Relu,
                                     scale=-1.0, bias=1.0)
                texp = tmpp.tile([C, 512], BF16, tag="texp")
                nc.scalar.activation(texp[:], tmin[:],
                                     mybir.ActivationFunctionType.Exp,
                                     scale=-1.0)
                nc.vector.tensor_max(dst[m][:, cs], texp[:], tu[:])

            # ---------------- P1b: l-major k via PE transposes ----------------
            # knat[lt] (128 l, 256 f); two (64,128)->(128,64) transposes per
            # (lt, 128-half) land in one (128,128) bf16 PSUM tile, one copy out.
            knat = [projp.tile([C, FPC], BF16, tag=f"kn{lt}", name=f"kn{lt}")
                    for lt in range(NCH)]

            def knat_T(lt):
                cs = slice(lt * C, (lt + 1) * C)
                for m in range(2):
                    pst = ps_t.tile([C, C], BF16, tag="t")
                    nc.tensor.matmul(pst[:], kp_h[m][:, cs], ident[:],
                                     is_transpose=True, start=True, stop=True)
                    dst = knat[lt][:, m * C:(m + 1) * C]
                    if (lt + m) % 2 == 0:
                        nc.vector.tensor_copy(dst, pst[:])
                    else:
                        nc.scalar.copy(dst, pst[:])

            # PE pre-warm: dep-free junk matmuls ramp the tensor engine to
            # full clock while the first x chunks stream in.
            import os as _os
            n_warm = int(_os.environ.get("TRN_PE_PREWARM", "20"))
            for i in range(n_warm):
                wps = ps_a.tile([C, C], F32, tag="a", name=f"warm{i}")
                nc.tensor.matmul(wps[:], ident[:], ident[:],
                                 start=True, stop=True)

            # k' fully; q' regions interleaved with knat transposes so PE
            # fills the k'-phi / xq-arrival latency.
            for m in range(2):
                for nbk in range(2):
                    proj_phi("k", wk_sb, xk_sb, kp_h, m, nbk)
            proj_phi("q", wq_sb, xq_sb, qp_h, 0, 0)
            for lt in range(4):
                knat_T(lt)
            proj_phi("q", wq_sb, xq_sb, qp_h, 0, 1)
            for lt in range(4, 8):
                knat_T(lt)
            proj_phi("q", wq_sb, xq_sb, qp_h, 1, 0)
            proj_phi("q", wq_sb, xq_sb, qp_h, 1, 1)

            # ---------------- P2 state/tiles shared defs ----------------
            # prefix tiles per pair, order [t0,u01,t2,u03,t4,u45,t6]; chunk
            # ranges and the per-chunk prefix cover:
            SB_GROUPS = [(0, 1), (0, 2), (2, 3), (0, 4), (4, 5), (4, 6),
                         (6, 7)]
            TERMS = {0: [], 1: [0], 2: [1], 3: [1, 2],
                     4: [3], 5: [3, 4], 6: [3, 5], 7: [3, 5, 6]}
            # two-pair tiles: pair 2m at partitions [0:64), 2m+1 at [64:128)
            # so odd pairs' prefix matmuls read lhsT/rhs at the same base.
            sb_h = [projp.tile([C, 7, 65], BF16, tag=f"sb{m}", name=f"sb{m}")
                    for m in range(2)]
            sb_ps = {}

            def sb_sl(p, ti):
                b = (p % 2) * D
                return sb_h[p // 2][b:b + D, ti, :]

            def sblock_mms(p, lo, hi):
                kcols = slice(p * D, (p + 1) * D)
                vcols = slice(p * 65, p * 65 + 65)
                m = p // 2
                if m not in sb_ps:
                    sb_ps[m] = ps_sh.tile([C, 7, 65], F32, tag="sh",
                                          name=f"sh{m}")
                b = (p % 2) * D
                ups = sb_ps[m]
                for ti in range(lo, hi):
                    c0, c1 = SB_GROUPS[ti]
                    for c in range(c0, c1):
                        nc.tensor.matmul(ups[b:b + D, ti, :],
                                         knat[c][:, kcols],
                                         vnat[c][:, vcols],
                                         start=(c == c0), stop=(c == c1 - 1))

            def sblock_copy(p, lo, hi, eng):
                m, b = p // 2, (p % 2) * D
                if eng == 0:
                    nc.vector.tensor_copy(sb_h[m][b:b + D, lo:hi, :],
                                          sb_ps[m][b:b + D, lo:hi, :])
                else:
                    nc.scalar.copy(sb_h[m][b:b + D, lo:hi, :],
                                   sb_ps[m][b:b + D, lo:hi, :])

            def emit_A(p, cg):
                """A'[s,t] for 4 chunks side by side, masked to bf16."""
                aps = ps_a.tile([C, 4 * C], F32, tag="a")
                for h in range(4):
                    cI = 4 * cg + h
                    cs = slice(cI * C, (cI + 1) * C)
                    nc.tensor.matmul(aps[:, h * C:(h + 1) * C],
                                     kp_sl(p, cs), qp_sl(p, cs),
                                     start=True, stop=True)
                asb = p2p.tile([C, 4 * C], BF16, tag="asb", bufs=8)
                nc.vector.tensor_mul(asb[:], aps[:], mask2[:])
                asbs[(p, cg)] = asb

            # ---------------- A2A buffers (l-major payload) ----------------
            # shard j = (128 l of chunk j, my 256 features)
            a2a_in = dram.tile([N_CORES * C, FPC], BF16, tag="a2ain")
            a2a_out = dram.tile([N_CORES * C, FPC], BF16, tag="a2aout")

            # ---------------- P2: chunked causal linear attention ----------
            def emit_O(p, cg, asb):
                """O'^T (128 l, 65) per chunk: A'^T V^ + Q'^T S_prefix."""
                vcols = slice(p * 65, p * 65 + 65)
                oT = ps_a.tile([C, 4, 65], F32, tag="a")
                for h in range(4):
                    cI = 4 * cg + h
                    cs = slice(cI * C, (cI + 1) * C)
                    tl = TERMS[cI]
                    nc.tensor.matmul(oT[:, h, :], asb[:, h * C:(h + 1) * C],
                                     vnat[cI][:, vcols],
                                     start=True, stop=not tl)
                    for i2, ti in enumerate(tl):
                        nc.tensor.matmul(oT[:, h, :], qp_sl(p, cs),
                                         sb_sl(p, ti),
                                         start=False, stop=(i2 == len(tl) - 1))
                return oT

            def emit_recip(p, oT):
                rcp = p2p.tile([C, 4, 1], F32, tag="rcp", bufs=4)
                nc.vector.reciprocal(rcp[:], oT[:, :, 64:65])
                return rcp

            def emit_div1(p, oT, rcp, h, att_sb, eng):
                dst = att_sb[h][:, p * D:(p + 1) * D]
                if eng == 0:
                    nc.vector.tensor_scalar(dst, oT[:, h, 0:64],
                                            rcp[:, h, :], None,
                                            op0=mybir.AluOpType.mult)
                else:
                    nc.scalar.activation(dst, oT[:, h, 0:64],
                                         mybir.ActivationFunctionType.Copy,
                                         scale=rcp[:, h, :])

            att_last = {}

            def emit_stage(cI, att_sb, h):
                """DMA one divided l-major chunk into its a2a_in shard."""
                nc.sync.dma_start(
                    out=a2a_in[cI * C:(cI + 1) * C, :],
                    in_=att_sb[h][:])

            oTs_all, rcps_all, att_all = {}, {}, {}

            def emit_cg_oT(cg):
                # att_sb[h]: divided attn chunk (128 l, 256 f)
                att_sb = [p2p.tile([C, FPC], BF16, tag=f"att{h}", bufs=2,
                                   name=f"att{cg}_{h}") for h in range(4)]
                att_last[cg] = att_sb
                att_all[cg] = att_sb
                oT0 = emit_O(0, cg, asbs[(0, cg)])
                oT1 = emit_O(1, cg, asbs[(1, cg)])
                rcp0 = emit_recip(0, oT0)
                oT2 = emit_O(2, cg, asbs[(2, cg)])
                rcp1 = emit_recip(1, oT1)
                oT3 = emit_O(3, cg, asbs[(3, cg)])
                rcp2 = emit_recip(2, oT2)
                rcp3 = emit_recip(3, oT3)
                oTs_all[cg] = [oT0, oT1, oT2, oT3]
                rcps_all[cg] = [rcp0, rcp1, rcp2, rcp3]

            def emit_cg_drain(cg):
                att_sb = att_all[cg]
                oTs, rcps = oTs_all[cg], rcps_all[cg]
                for h in range(4):
                    for p in range(4):
                        emit_div1(p, oTs[p], rcps[p], h, att_sb,
                                  (p + h) % 2)
                    emit_stage(4 * cg + h, att_sb, h)

            asbs = {}
            # ---------------- P1c: natural-layout v (ones col), interleaved
            # with A' (fills xv-arrival gaps) and the prefix-state blocks.
            vnat = []
            for lt in range(NCH):
                vt = projp.tile([C, 4 * 65], BF16, tag=f"vn{lt}")
                nc.vector.memset(
                    vt[:].rearrange("p (f c) -> p f c", f=4)[:, :, 64:65], 1.0)
                ps = ps_pj.tile([C, FPC], F32, tag="pj")
                for k in range(8):
                    nc.tensor.matmul(ps[:], xv_sb[k][:, lt * C:(lt + 1) * C],
                                     wv_sb[k][:], start=(k == 0), stop=(k == 7))
                vna = vt[:].rearrange("p (f c) -> p f c", f=4)
                if with_bias:
                    nc.vector.tensor_add(vna[:, :, 0:64],
                                         ps[:].rearrange("p (f c) -> p f c", f=4),
                                         bv_bc[:].rearrange("p (f c) -> p f c", f=4))
                else:
                    nc.scalar.copy(vna[:, :, 0:64],
                                   ps[:].rearrange("p (f c) -> p f c", f=4))
                vnat.append(vt)
                if lt == 1:
                    emit_A(0, 0)
                    emit_A(1, 0)
                elif lt == 2:
                    emit_A(2, 0)
                    emit_A(3, 0)
                elif lt == 3:
                    for p in range(4):
                        sblock_mms(p, 0, 4)
                        sblock_copy(p, 0, 4, p % 2)
                    emit_A(0, 1)
                    emit_A(1, 1)
                elif lt == 5:
                    emit_A(2, 1)
                    emit_A(3, 1)
                    emit_cg_oT(0)
            for p in range(4):
                sblock_mms(p, 4, 7)
                sblock_copy(p, 4, 7, p % 2)
            emit_cg_oT(1)
            emit_cg_drain(0)
            emit_cg_drain(1)
            pj_stack.close()


            # PE keeper: a self-paced transpose/copy chain spanning the
            # collective window so the tensor engine p-state stays ramped
            # for P3. Seeded off the last attnT writes.
            n_links = int(_os.environ.get("TRN_PE_KEEPER", "42"))
            if n_links:
                scr = [p2p.tile([C, C], BF16, tag=f"scr{i}", bufs=1,
                                name=f"scr{i}") for i in range(2)]
                nc.vector.tensor_copy(scr[0][:], att_last[1][3][:, 0:C])
                for i in range(n_links):
                    pst = ps_t.tile([C, C], BF16, tag="t", name=f"keep{i}")
                    nc.tensor.matmul(pst[:], scr[i % 2][:], ident[:],
                                     is_transpose=True, start=True, stop=True)
                    if i % 2 == 0:
                        nc.vector.tensor_copy(scr[(i + 1) % 2][:], pst[:])
                    else:
                        nc.scalar.copy(scr[(i + 1) % 2][:], pst[:])

            # ---------------- A2A: single 8-way AllToAll ----------------
            nc.gpsimd.collective_compute(
                "AllToAll",
                mybir.AluOpType.bypass,
                replica_groups=[list(range(N_CORES))],
                ins=[a2a_in.opt()],
                outs=[a2a_out.opt()],
            )

            # ---------------- P3: output projection ----------------
            ps_3 = es.enter_context(tc.tile_pool(name="ps_3", bufs=2,
                                                 space="PSUM"))
            athL, ath = {}, {}
            for n in range(NB):
                t = atTp.tile([C, 4, FPC], BF16, tag=f"athL{n}",
                              name=f"athL{n}")
                for hf in range(2):
                    nc.sync.dma_start(
                        out=t[:, 2 * hf:2 * (hf + 1), :],
                        in_=a2a_out[(2 * n + hf) * 2 * C:
                                    (2 * n + hf + 1) * 2 * C, :]
                            .rearrange("(s p) f -> p s f", p=C))
                athL[n] = t
            for n in range(NB):
                tf = atTp.tile([C, 8, C], BF16, tag=f"ath{n}", name=f"ath{n}")
                for s in range(4):
                    for half in range(2):
                        pst = ps_t.tile([C, C], BF16, tag="t",
                                        name=f"pT{n}_{s}_{half}")
                        nc.tensor.matmul(
                            pst[:],
                            athL[n][:, s, half * C:(half + 1) * C],
                            ident[:], is_transpose=True,
                            start=True, stop=True)
                        j = 2 * s + half
                        if j % 2 == 0:
                            nc.vector.tensor_copy(tf[:, j, :], pst[:])
                        else:
                            nc.scalar.copy(tf[:, j, :], pst[:])
                ath[n] = tf
            for n in range(NB):
                for nbk in range(2):
                    cs = slice(nbk * 512, (nbk + 1) * 512)
                    ps3 = ps_3.tile([C, 512], F32, tag="p3")
                    for j in range(8):
                        nc.tensor.matmul(ps3[:], ath[n][:, j, :],
                                         wo_sb[j][:, cs],
                                         start=(j == 0), stop=(j == 7))
                    osb = outp.tile([C, 512], F32, tag=f"osb{n}{nbk}",
                                    name=f"osb{n}{nbk}")
                    if with_bias:
                        nc.vector.tensor_add(osb[:], ps3[:], bo_bc[:, cs])
                    elif (n + nbk) % 2 == 0:
                        nc.vector.tensor_copy(osb[:], ps3[:])
                    else:
                        nc.scalar.copy(osb[:], ps3[:])
                    nc.sync.dma_start(out=out_d[n][:, cs], in_=osb[:])
            nc.sync.dma_start(out=tok_o[:], in_=tok_i[:])

    _split_waits(nc)
    return nc


def _run_pjrt_timed(nc, in_maps, time_iters=0):
    """Replicates bass2jax.run_bass_via_pjrt's multi-core path, but keeps
    inputs device-resident and (optionally) times repeated executions.
    Returns (results, best_exec_seconds_or_None)."""
    import time as _time
    import jax
    from jax.sharding import Mesh, PartitionSpec, NamedSharding
    from jax.experimental.shard_map import shard_map
    from concourse import bass2jax, mybir as mb

    bass2jax.install_neuronx_cc_hook()
    n_cores = len(in_maps)
    partition_name = nc.partition_id_tensor.name if nc.partition_id_tensor else None

    in_names, out_names, out_avals, zero_outs = [], [], [], []
    for alloc in nc.m.functions[0].allocations:
        if not isinstance(alloc, mb.MemoryLocationSet):
            continue
        name = alloc.memorylocations[0].name
        if alloc.kind == "ExternalInput":
            if name != partition_name:
                in_names.append(name)
        elif alloc.kind == "ExternalOutput":
            out_names.append(name)
            shape = tuple(alloc.tensor_shape)
            dtype = mb.dt.np(alloc.dtype)
            out_avals.append(jax.core.ShapedArray(shape, dtype))
            zero_outs.append(np.zeros(shape, dtype))
    n_params = len(in_names)
    in_names.extend(out_names)
    if partition_name is not None:
        in_names.append(partition_name)

    chain = int(__import__("os").environ.get("TRN_KERNEL_CHAIN", "1"))
    tok_in_idx = in_names.index("tok") if "tok" in in_names else None
    tok_out_idx = out_names.index("tok_out") if "tok_out" in out_names else None

    def _body(*args):
        operands = list(args)
        pid = bass2jax.partition_id_tensor() if partition_name is not None else None
        outs = None
        for _ in range(chain):
            ops = list(operands)
            if outs is not None and tok_in_idx is not None:
                ops[tok_in_idx] = outs[tok_out_idx]  # serialize iterations
            if pid is not None:
                ops.append(pid)
            outs = bass2jax._bass_exec_p.bind(
                *ops,
                out_avals=tuple(out_avals),
                in_names=tuple(in_names),
                out_names=tuple(out_names),
                lowering_input_output_aliases=(),
                sim_require_finite=True,
                sim_require_nnan=True,
                nc=nc,
            )
        return tuple(outs)

    devices = jax.devices()[:n_cores]
    mesh = Mesh(np.asarray(devices), ("core",))
    in_specs = (PartitionSpec("core"),) * (n_params + len(out_names))
    out_specs = (PartitionSpec("core"),) * len(out_names)
    sharded = jax.jit(
        shard_map(_body, mesh=mesh, in_specs=in_specs, out_specs=out_specs,
                  check_rep=False),
        keep_unused=True,
    )
    per_core = [[np.asarray(m[name]) for name in in_names[:n_params]]
                for m in in_maps]
    concat_in = [np.concatenate([per_core[c][i] for c in range(n_cores)], axis=0)
                 for i in range(n_params)]
    concat_zeros = [np.zeros((n_cores * z.shape[0], *z.shape[1:]), z.dtype)
                    for z in zero_outs]
    shd = NamedSharding(mesh, PartitionSpec("core"))
    dev_in = [jax.device_put(a, shd) for a in concat_in + concat_zeros]

    out_arrs = sharded(*dev_in)
    jax.block_until_ready(out_arrs)
    best = None
    for _ in range(time_iters):
        t0 = _time.perf_counter()
        out_arrs2 = sharded(*dev_in)
        jax.block_until_ready(out_arrs2)
        dt = _time.perf_counter() - t0
        best = dt if best is None or dt < best else best
    results = [
        {name: np.asarray(out_arrs[i]).reshape(n_cores, *out_avals[i].shape)[c]
         for i, name in enumerate(out_names)}
        for c in range(n_cores)
    ]
    return results, best


def kernel(**inputs):
    global LAST_RESULT
    import os

    query = np.asarray(inputs["query"], np.float32)
    key = np.asarray(inputs["key"], np.float32)
    value = np.asarray(inputs["value"], np.float32)
    Wq = np.asarray(inputs["Wq"], np.float32)
    Wk = np.asarray(inputs["Wk"], np.float32)
    Wv = np.asarray(inputs["Wv"], np.float32)
    Wo = np.asarray(inputs["Wo"], np.float32)
    bq = np.asarray(inputs["bq"], np.float32)
    bk = np.asarray(inputs["bk"], np.float32)
    bv = np.asarray(inputs["bv"], np.float32)
    bo = np.asarray(inputs["bo"], np.float32)

    with_bias = any(np.any(b) for b in (bq, bk, bv, bo))
    nc = _build(with_bias)

    woT_full = np.ascontiguousarray(Wo.T)
    triu = np.triu(np.ones((C, C), np.float32))   # mask[s,t]=1 iff s<=t
    mask_c = np.ascontiguousarray(np.concatenate([triu] * 4, axis=1))
    import ml_dtypes
    ident_c = np.eye(C).astype(ml_dtypes.bfloat16)
    in_maps = []
    for c in range(N_CORES):
        a, b = c // 4, c % 4
        F = slice(FPC * b, FPC * (b + 1))
        m = {
            "xqT": np.ascontiguousarray(query[:, a, :].T),
            "xkT": np.ascontiguousarray(key[:, a, :].T),
            "xvT": np.ascontiguousarray(value[:, a, :].T),
            "wqT": np.ascontiguousarray(Wq[F, :].T),
            "wkT": np.ascontiguousarray(Wk[F, :].T),
            "wvT": np.ascontiguousarray(Wv[F, :].T),
            "woT": woT_full,
            "maskc": mask_c,
            "identc": ident_c,
            "tok": np.zeros((1, 1), np.float32),
        }
        if with_bias:
            m["bq"] = np.ascontiguousarray(bq[F].reshape(FPC, 1))
            m["bk"] = np.ascontiguousarray(bk[F].reshape(FPC, 1))
            m["bv"] = np.ascontiguousarray(bv[F].reshape(FPC, 1))
            m["bo"] = np.ascontiguousarray(bo.reshape(1, E))
        in_maps.append(m)

    time_iters = int(os.environ.get("TRN_KERNEL_TIME_ITERS", "0"))
    results, best = _run_pjrt_timed(nc, in_maps, time_iters=time_iters)
    LAST_RESULT = {"results": results, "best_exec_s": best}

    out = np.empty((L, NB, E), np.float32)
    for c in range(N_CORES):
        o = results[c]["out"]  # (NB, C, E): my l-chunk rows for both batches
        for n in range(NB):
            out[c * C:(c + 1) * C, n, :] = o[n]
    return out
